# revision 16
# baseline (speedup 1.0000x reference)
"""LookAheadMask kernel for Trainium2 — in-place, merged diag writes.

out[b, r, c] = 1.0 if c > r else x[b, r, c], for x of shape (8, 4096, 4096) f32.

Sharding: batch dim across 8 NeuronCores (data parallel, no communication).

The output aliases the input buffer (lowering_input_output_aliases={0: 0}
through the BIR-lowering/NKI path), so the strictly-lower triangle never
moves. Per-core work is ~4 MiB of HBM reads + ~34 MiB of HBM writes.

Measured DMA-head behavior (v1/v2 traces): big-descriptor writes stream at
~430-470 GB/s per HWDGE ring; small-descriptor DMAs are head-limited at
~6-8 ns/desc on the SP ring but ~23 ns/desc on the ACT ring, and a
512 B-descriptor scatter costs ~25-60 us wherever it runs. So:

  - The 1 KiB-descriptor diag gather (4096 descs, unavoidable: the diag
    band is 4096 scattered 512 B row segments) runs entirely on the SP
    ring, split in two chunks to pipeline the selects.
  - There is NO scatter. A [128, 32*1024] SBUF tile (diag_sel) is
    pre-memset to 1.0 (split across DVE and gpsimd); gpsimd affine_selects
    only the 128-wide diagonal columns of each block into it; each diag
    block then leaves SBUF as the leading 128 cols of a [128 x 1024]
    4 KiB-descriptor rectangle (byte-bound, not desc-bound).
  - Pure-ones rectangles cover cols >= blockstart+1024 from a [128, 3072]
    ones tile; blocks 24-31 are fully covered by the (clipped) merged
    rectangles.
  - Two pure-ones rectangles go through the gpsimd SWDGE queue to measure
    a third DMA head; the rest are balanced SP/ACT.
"""

import numpy as np

S = 4096
P = 128
NB = S // P  # 32
N_CORES = 8
W = 256  # diag gather window width (1 KiB descriptors)
MW = 1024  # merged diag-rectangle width (4 KiB descriptors)
DB = P * S + P  # element stride between consecutive diagonal blocks

D2D_BLOCKS = [0, 1, 2, 3, 4, 5]  # pure rects sourced from DRAM (bypass SBUF)
ACT_BLOCKS = [i for i in range(24) if i not in D2D_BLOCKS]

_cached = None


def _build():
    from concourse import bass, mybir

    nc = bass.Bass(target_bir_lowering=True, enable_partition_id=False)
    x = nc.dram_tensor("x", [S, S], mybir.dt.float32, kind="ExternalInput")
    out = nc.dram_tensor("out", [S, S], mybir.dt.float32, kind="ExternalOutput")
    # DRAM ones scratch: D2D-sourced rectangles read it instead of SBUF,
    # relieving the 435 GB/s SBUF AXI fabric (the measured bottleneck:
    # 38 MiB of SBUF-side DMA bytes / 435 GB/s + 13 us overhead = 105 us).
    dones = nc.dram_tensor("dones", [P, 3072], mybir.dt.float32, kind="Internal")

    N_WRITES = 24 + 2 + 8  # pure ones + merged chunks + clipped blocks

    def pure_ones(eng, blocks, ones, dsem):
        for i in blocks:
            r0 = i * P
            w = S - r0 - MW
            eng.dma_start(
                out=out[r0 : r0 + P, r0 + MW : S], in_=ones[:, :w]
            ).then_inc(dsem, 16)

    with (
        nc.Block() as block,
        nc.semaphore("dsem") as dsem,  # all output-write DMA completions
        nc.semaphore("gsa") as gsa,  # gather chunks (SP ring)
        nc.semaphore("msem") as msem,  # ones memset done
        nc.semaphore("m2") as m2,  # diag_sel DVE-half memset done
        nc.semaphore("asem") as asem,  # affine_selects done
        nc.semaphore("zsem") as zsem,  # DRAM ones scratch filled
        nc.sbuf_tensor("ones", [P, S - MW], mybir.dt.float32) as ones,
        nc.sbuf_tensor("diag_in2", [P, NB * W], mybir.dt.float32) as diag_in2,
        nc.sbuf_tensor("diag_sel", [P, NB * MW], mybir.dt.float32) as diag_sel,
    ):

        @block.vector
        def _(vector: bass.BassVectorEngine):
            vector.memset(ones[:, :], 1.0).then_inc(msem, 1)
            vector.memset(diag_sel[:, : 16 * MW], 1.0).then_inc(m2, 1)

        @block.sync
        def _(sync: bass.BassEngine):
            # Diag gather, 1 KiB descriptors, all on the fast SP head.
            # Block 0's window would start before the tensor: own 128-col load.
            sync.dma_start(
                out=bass.AP(diag_in2, W - P, [[NB * W, P], [1, P]]),
                in_=x[0:P, 0:P],
            ).then_inc(gsa, 16)
            sync.dma_start(
                out=bass.AP(diag_in2, W, [[NB * W, P], [W, 15], [1, W]]),
                in_=bass.AP(x, DB + P - W, [[S, P], [DB, 15], [1, W]]),
            ).then_inc(gsa, 16)
            sync.dma_start(
                out=bass.AP(diag_in2, 16 * W, [[NB * W, P], [W, 16], [1, W]]),
                in_=bass.AP(x, 16 * DB + P - W, [[S, P], [DB, 16], [1, W]]),
            ).then_inc(gsa, 16)
            # Hold SP's byte work until its gather descriptors fully
            # drain: bytes queued on this ring collapse the gather's
            # drain rate from ~8 to ~16-22 ns/desc (v4/v5 traces).
            sync.wait_ge(gsa, 48)
            sync.wait_ge(zsem, 16)
            for i in D2D_BLOCKS:
                r0 = i * P
                w = S - r0 - MW
                sync.dma_start(
                    out=out[r0 : r0 + P, r0 + MW : S], in_=dones[:, :w]
                ).then_inc(dsem, 16)
            sync.wait_ge(asem, 2)
            # Blocks 24-31: merged rect clipped at the right edge covers the
            # whole remaining row span [r0, S). Narrow descriptors cost
            # ~8 ns each on SP vs ~19 ns on ACT, and SP idles here anyway.
            for b in range(24, 32):
                r0 = b * P
                w = S - r0
                sync.dma_start(
                    out=out[r0 : r0 + P, r0:S],
                    in_=bass.AP(diag_sel, b * MW, [[NB * MW, P], [1, w]]),
                ).then_inc(dsem, 16)
            sync.wait_ge(dsem, 16 * N_WRITES)

        @block.scalar
        def _(scalar: bass.BassEngine):
            # ACT streams once gather chunk 1 (blocks 0-15) is done — a
            # compromise between ring utilization and gather drain rate.
            scalar.wait_ge(gsa, 32)
            scalar.wait_ge(msem, 1)
            scalar.dma_start(out=dones[:, :], in_=ones[:, :3072]).then_inc(
                zsem, 16
            )
            pure_ones(scalar, ACT_BLOCKS[:8], ones, dsem)
            scalar.wait_ge(asem, 1)
            # Merged rectangles for diag blocks 0-15: [128 x 1024] each,
            # leading 128 cols are the selected diag, rest ones.
            scalar.dma_start(
                out=bass.AP(out, 0, [[S, P], [DB, 16], [1, MW]]),
                in_=bass.AP(diag_sel, 0, [[NB * MW, P], [MW, 16], [1, MW]]),
            ).then_inc(dsem, 16)
            pure_ones(scalar, ACT_BLOCKS[8:], ones, dsem)
            scalar.wait_ge(asem, 2)
            # Merged rectangles for diag blocks 16-23.
            scalar.dma_start(
                out=bass.AP(out, 16 * DB, [[S, P], [DB, 8], [1, MW]]),
                in_=bass.AP(
                    diag_sel, 16 * MW, [[NB * MW, P], [MW, 8], [1, MW]]
                ),
            ).then_inc(dsem, 16)

        @block.gpsimd
        def _(gpsimd: bass.BassGpSimd):
            gpsimd.memset(diag_sel[:, 16 * MW :], 1.0)
            # iota[p, c] = p - (c % 128); keep x where >= 0 (at/below diag).
            # Select ONLY the 128 diag cols of each 1024-wide window; the
            # other 896 cols stay at the memset 1.0.
            gpsimd.wait_ge(gsa, 32)  # block 0 + blocks 1-15
            gpsimd.wait_ge(m2, 1)
            gpsimd.affine_select(
                out=bass.AP(diag_sel, 0, [[NB * MW, P], [MW, 16], [1, P]]),
                in_=bass.AP(diag_in2, W - P, [[NB * W, P], [W, 16], [1, P]]),
                pattern=[[0, 16], [-1, P]],
                base=0,
                channel_multiplier=1,
                compare_op=mybir.AluOpType.is_ge,
                fill=1.0,
            ).then_inc(asem, 1)
            gpsimd.wait_ge(gsa, 48)  # blocks 16-31
            gpsimd.affine_select(
                out=bass.AP(
                    diag_sel, 16 * MW, [[NB * MW, P], [MW, 16], [1, P]]
                ),
                in_=bass.AP(
                    diag_in2, 16 * W + W - P, [[NB * W, P], [W, 16], [1, P]]
                ),
                pattern=[[0, 16], [-1, P]],
                base=0,
                channel_multiplier=1,
                compare_op=mybir.AluOpType.is_ge,
                fill=1.0,
            ).then_inc(asem, 1)

    nc.finalize()
    return nc


def _make_runner():
    """Compile-once runner: jit(shard_map(_body)) over 8 cores with the
    output aliased to the (donated) input — mirrors
    bass2jax.run_bass_via_pjrt, plus lowering_input_output_aliases."""
    global _cached
    if _cached is not None:
        return _cached

    import jax
    from jax.sharding import Mesh, PartitionSpec
    from jax.experimental.shard_map import shard_map
    from concourse import bass2jax

    bass2jax.install_neuronx_cc_hook()
    nc = _build()

    def _body(xg):
        outs = bass2jax._bass_exec_p.bind(
            xg,
            out_avals=(jax.core.ShapedArray((S, S), np.float32),),
            in_names=("x",),
            out_names=("out",),
            lowering_input_output_aliases=((0, 0),),
            sim_require_finite=True,
            sim_require_nnan=True,
            nc=nc,
        )
        return tuple(outs)

    devices = jax.devices()[:N_CORES]
    assert len(devices) == N_CORES, f"need {N_CORES} devices, have {len(devices)}"
    mesh = Mesh(np.asarray(devices), ("core",))
    sharded = jax.jit(
        shard_map(
            _body,
            mesh=mesh,
            in_specs=(PartitionSpec("core"),),
            out_specs=(PartitionSpec("core"),),
            check_rep=False,
        ),
        donate_argnums=(0,),
        keep_unused=True,
    )
    _cached = (nc, sharded)
    return _cached


class _Result:
    def __init__(self, exec_time_ns=None, mean_exec_time_ns=None):
        self.exec_time_ns = exec_time_ns
        self.mean_exec_time_ns = mean_exec_time_ns


def _run(x_full: np.ndarray, trace: bool = False):
    nc, sharded = _make_runner()
    x_full = np.asarray(x_full, dtype=np.float32)
    xg = np.ascontiguousarray(x_full.reshape(N_CORES * S, S))

    if not trace:
        out = sharded(xg)[0]
        return np.asarray(out).reshape(N_CORES, S, S), _Result()

    # Trace path (test.py only): NTFF profile around the execution, then the
    # same gauge/perfetto pipeline run_bass_kernel_spmd uses under axon.
    import glob
    import os
    import tempfile

    from antenv.axon_hooks import get_axon_ntff_profile_hook
    from concourse import bass_utils as BU

    neff_dir = tempfile.mkdtemp()
    hook = get_axon_ntff_profile_hook()
    with hook(neff_dir, [0]):
        out = np.asarray(sharded(xg)[0])

    ntffs = glob.glob(os.path.join(neff_dir, "*_body*.ntff"))
    if not ntffs:
        return out.reshape(N_CORES, S, S), _Result()

    sharepath = BU.upload_artifacts(neff_dir)
    profile = BU.gauge.profiler.Profile(
        profile_path=BU.FishPath(neff_dir),
        kernel_dev_mode=True,
        profile_on_exit=False,
        bass_kernel=nc.m,
        offline_processing=True,
        fname="*_body*",
        annotate_hlo=False,
        metadata={"artifacts_path": sharepath},
    )
    perf = BU._process_ntff_profile(
        profile,
        neff_dir,
        nc,
        list(range(N_CORES)),
        None,
        False,
        {},
        trace_events=False,
    )
    return out.reshape(N_CORES, S, S), _Result(
        perf.exec_time_ns, perf.mean_exec_time_ns
    )


def kernel(x: np.ndarray) -> np.ndarray:
    out, _ = _run(x, trace=False)
    return out


# revision 17
# speedup vs baseline: 1.0503x; 1.0503x over previous
"""LookAheadMask kernel for Trainium2 — in-place, merged diag writes.

out[b, r, c] = 1.0 if c > r else x[b, r, c], for x of shape (8, 4096, 4096) f32.

Sharding: batch dim across 8 NeuronCores (data parallel, no communication).

The output aliases the input buffer (lowering_input_output_aliases={0: 0}
through the BIR-lowering/NKI path), so the strictly-lower triangle never
moves. Per-core work is ~4 MiB of HBM reads + ~34 MiB of HBM writes.

Measured DMA-head behavior (v1/v2 traces): big-descriptor writes stream at
~430-470 GB/s per HWDGE ring; small-descriptor DMAs are head-limited at
~6-8 ns/desc on the SP ring but ~23 ns/desc on the ACT ring, and a
512 B-descriptor scatter costs ~25-60 us wherever it runs. So:

  - The 1 KiB-descriptor diag gather (4096 descs, unavoidable: the diag
    band is 4096 scattered 512 B row segments) runs entirely on the SP
    ring, split in two chunks to pipeline the selects.
  - There is NO scatter. A [128, 32*1024] SBUF tile (diag_sel) is
    pre-memset to 1.0 (split across DVE and gpsimd); gpsimd affine_selects
    only the 128-wide diagonal columns of each block into it; each diag
    block then leaves SBUF as the leading 128 cols of a [128 x 1024]
    4 KiB-descriptor rectangle (byte-bound, not desc-bound).
  - Pure-ones rectangles cover cols >= blockstart+1024 from a [128, 3072]
    ones tile; blocks 24-31 are fully covered by the (clipped) merged
    rectangles.
  - Two pure-ones rectangles go through the gpsimd SWDGE queue to measure
    a third DMA head; the rest are balanced SP/ACT.
"""

import numpy as np

S = 4096
P = 128
NB = S // P  # 32
N_CORES = 8
W = 256  # diag gather window width (1 KiB descriptors)
MW = 1024  # merged diag-rectangle width (4 KiB descriptors)
DB = P * S + P  # element stride between consecutive diagonal blocks

SP_EARLY = [0]  # pure rect issued on SP once its gathers fully drain
SP_LATE = [1, 2, 3, 4]  # pure rects issued on SP after the clipped blocks
ACT_BLOCKS = [i for i in range(24) if i not in SP_EARLY + SP_LATE]

_cached = None


def _build():
    from concourse import bass, mybir

    nc = bass.Bass(target_bir_lowering=True, enable_partition_id=False)
    x = nc.dram_tensor("x", [S, S], mybir.dt.float32, kind="ExternalInput")
    out = nc.dram_tensor("out", [S, S], mybir.dt.float32, kind="ExternalOutput")

    N_WRITES = 24 + 2 + 8  # pure ones + merged chunks + clipped blocks

    def pure_ones(eng, blocks, ones, dsem):
        for i in blocks:
            r0 = i * P
            w = S - r0 - MW
            eng.dma_start(
                out=out[r0 : r0 + P, r0 + MW : S], in_=ones[:, :w]
            ).then_inc(dsem, 16)

    with (
        nc.Block() as block,
        nc.semaphore("dsem") as dsem,  # all output-write DMA completions
        nc.semaphore("gsa") as gsa,  # gather chunks (SP ring)
        nc.semaphore("msem") as msem,  # ones memset done
        nc.semaphore("m2") as m2,  # diag_sel DVE-half memset done
        nc.semaphore("asem") as asem,  # affine_selects done
        nc.sbuf_tensor("ones", [P, S - MW], mybir.dt.float32) as ones,
        nc.sbuf_tensor("diag_in2", [P, NB * W], mybir.dt.float32) as diag_in2,
        nc.sbuf_tensor("diag_sel", [P, NB * MW], mybir.dt.float32) as diag_sel,
    ):

        @block.vector
        def _(vector: bass.BassVectorEngine):
            vector.memset(ones[:, :], 1.0).then_inc(msem, 1)
            vector.memset(diag_sel[:, : 16 * MW], 1.0).then_inc(m2, 1)

        @block.sync
        def _(sync: bass.BassEngine):
            # Diag gather, 1 KiB descriptors, all on the fast SP head.
            # Block 0's window would start before the tensor: own 128-col load.
            sync.dma_start(
                out=bass.AP(diag_in2, W - P, [[NB * W, P], [1, P]]),
                in_=x[0:P, 0:P],
            ).then_inc(gsa, 16)
            sync.dma_start(
                out=bass.AP(diag_in2, W, [[NB * W, P], [W, 15], [1, W]]),
                in_=bass.AP(x, DB + P - W, [[S, P], [DB, 15], [1, W]]),
            ).then_inc(gsa, 16)
            sync.dma_start(
                out=bass.AP(diag_in2, 16 * W, [[NB * W, P], [W, 16], [1, W]]),
                in_=bass.AP(x, 16 * DB + P - W, [[S, P], [DB, 16], [1, W]]),
            ).then_inc(gsa, 16)
            # Hold SP's byte work until its gather descriptors fully
            # drain: bytes queued on this ring collapse the gather's
            # drain rate from ~8 to ~16-22 ns/desc (v4/v5 traces).
            sync.wait_ge(gsa, 48)
            sync.wait_ge(msem, 1)
            pure_ones(sync, SP_EARLY, ones, dsem)
            sync.wait_ge(asem, 2)
            # Blocks 24-31: merged rect clipped at the right edge covers the
            # whole remaining row span [r0, S). Narrow descriptors cost
            # ~8 ns each on SP vs ~19 ns on ACT, and SP idles here anyway.
            for b in range(24, 32):
                r0 = b * P
                w = S - r0
                sync.dma_start(
                    out=out[r0 : r0 + P, r0:S],
                    in_=bass.AP(diag_sel, b * MW, [[NB * MW, P], [1, w]]),
                ).then_inc(dsem, 16)
            pure_ones(sync, SP_LATE, ones, dsem)
            sync.wait_ge(dsem, 16 * N_WRITES)

        @block.scalar
        def _(scalar: bass.BassEngine):
            # ACT streams once gather chunk 1 (blocks 0-15) is done — a
            # compromise between ring utilization and gather drain rate.
            scalar.wait_ge(gsa, 32)
            scalar.wait_ge(msem, 1)
            pure_ones(scalar, ACT_BLOCKS[:8], ones, dsem)
            scalar.wait_ge(asem, 1)
            # Merged rectangles for diag blocks 0-15: [128 x 1024] each,
            # leading 128 cols are the selected diag, rest ones.
            scalar.dma_start(
                out=bass.AP(out, 0, [[S, P], [DB, 16], [1, MW]]),
                in_=bass.AP(diag_sel, 0, [[NB * MW, P], [MW, 16], [1, MW]]),
            ).then_inc(dsem, 16)
            pure_ones(scalar, ACT_BLOCKS[8:], ones, dsem)
            scalar.wait_ge(asem, 2)
            # Merged rectangles for diag blocks 16-23.
            scalar.dma_start(
                out=bass.AP(out, 16 * DB, [[S, P], [DB, 8], [1, MW]]),
                in_=bass.AP(
                    diag_sel, 16 * MW, [[NB * MW, P], [MW, 8], [1, MW]]
                ),
            ).then_inc(dsem, 16)

        @block.gpsimd
        def _(gpsimd: bass.BassGpSimd):
            gpsimd.memset(diag_sel[:, 16 * MW :], 1.0)
            # iota[p, c] = p - (c % 128); keep x where >= 0 (at/below diag).
            # Select ONLY the 128 diag cols of each 1024-wide window; the
            # other 896 cols stay at the memset 1.0.
            gpsimd.wait_ge(gsa, 32)  # block 0 + blocks 1-15
            gpsimd.wait_ge(m2, 1)
            gpsimd.affine_select(
                out=bass.AP(diag_sel, 0, [[NB * MW, P], [MW, 16], [1, P]]),
                in_=bass.AP(diag_in2, W - P, [[NB * W, P], [W, 16], [1, P]]),
                pattern=[[0, 16], [-1, P]],
                base=0,
                channel_multiplier=1,
                compare_op=mybir.AluOpType.is_ge,
                fill=1.0,
            ).then_inc(asem, 1)
            gpsimd.wait_ge(gsa, 48)  # blocks 16-31
            gpsimd.affine_select(
                out=bass.AP(
                    diag_sel, 16 * MW, [[NB * MW, P], [MW, 16], [1, P]]
                ),
                in_=bass.AP(
                    diag_in2, 16 * W + W - P, [[NB * W, P], [W, 16], [1, P]]
                ),
                pattern=[[0, 16], [-1, P]],
                base=0,
                channel_multiplier=1,
                compare_op=mybir.AluOpType.is_ge,
                fill=1.0,
            ).then_inc(asem, 1)

    nc.finalize()
    return nc


def _make_runner():
    """Compile-once runner: jit(shard_map(_body)) over 8 cores with the
    output aliased to the (donated) input — mirrors
    bass2jax.run_bass_via_pjrt, plus lowering_input_output_aliases."""
    global _cached
    if _cached is not None:
        return _cached

    import jax
    from jax.sharding import Mesh, PartitionSpec
    from jax.experimental.shard_map import shard_map
    from concourse import bass2jax

    bass2jax.install_neuronx_cc_hook()
    nc = _build()

    def _body(xg):
        outs = bass2jax._bass_exec_p.bind(
            xg,
            out_avals=(jax.core.ShapedArray((S, S), np.float32),),
            in_names=("x",),
            out_names=("out",),
            lowering_input_output_aliases=((0, 0),),
            sim_require_finite=True,
            sim_require_nnan=True,
            nc=nc,
        )
        return tuple(outs)

    devices = jax.devices()[:N_CORES]
    assert len(devices) == N_CORES, f"need {N_CORES} devices, have {len(devices)}"
    mesh = Mesh(np.asarray(devices), ("core",))
    sharded = jax.jit(
        shard_map(
            _body,
            mesh=mesh,
            in_specs=(PartitionSpec("core"),),
            out_specs=(PartitionSpec("core"),),
            check_rep=False,
        ),
        donate_argnums=(0,),
        keep_unused=True,
    )
    _cached = (nc, sharded)
    return _cached


class _Result:
    def __init__(self, exec_time_ns=None, mean_exec_time_ns=None):
        self.exec_time_ns = exec_time_ns
        self.mean_exec_time_ns = mean_exec_time_ns


def _run(x_full: np.ndarray, trace: bool = False):
    nc, sharded = _make_runner()
    x_full = np.asarray(x_full, dtype=np.float32)
    xg = np.ascontiguousarray(x_full.reshape(N_CORES * S, S))

    if not trace:
        out = sharded(xg)[0]
        return np.asarray(out).reshape(N_CORES, S, S), _Result()

    # Trace path (test.py only): NTFF profile around the execution, then the
    # same gauge/perfetto pipeline run_bass_kernel_spmd uses under axon.
    import glob
    import os
    import tempfile

    from antenv.axon_hooks import get_axon_ntff_profile_hook
    from concourse import bass_utils as BU

    neff_dir = tempfile.mkdtemp()
    hook = get_axon_ntff_profile_hook()
    with hook(neff_dir, [0]):
        out = np.asarray(sharded(xg)[0])

    ntffs = glob.glob(os.path.join(neff_dir, "*_body*.ntff"))
    if not ntffs:
        return out.reshape(N_CORES, S, S), _Result()

    sharepath = BU.upload_artifacts(neff_dir)
    profile = BU.gauge.profiler.Profile(
        profile_path=BU.FishPath(neff_dir),
        kernel_dev_mode=True,
        profile_on_exit=False,
        bass_kernel=nc.m,
        offline_processing=True,
        fname="*_body*",
        annotate_hlo=False,
        metadata={"artifacts_path": sharepath},
    )
    perf = BU._process_ntff_profile(
        profile,
        neff_dir,
        nc,
        list(range(N_CORES)),
        None,
        False,
        {},
        trace_events=False,
    )
    return out.reshape(N_CORES, S, S), _Result(
        perf.exec_time_ns, perf.mean_exec_time_ns
    )


def kernel(x: np.ndarray) -> np.ndarray:
    out, _ = _run(x, trace=False)
    return out
